# revision 6
# baseline (speedup 1.0000x reference)
"""Trainium2 Bass kernel for single-head attention (N=16384, F=512, M=128),
sequence-parallel over 8 NeuronCores.

v2 strategy (hardcoded, self-contained):
- Each core owns 2048 query rows. K/V' projections are computed redundantly on
  every core (fp8 DoubleRow) -> no collectives.
- Wo is folded into V on the host: W2 = Wv @ Wo (fp8), so the kernel computes
  O^T = V'^T E^T directly with no output projection; bv@Wo + bo folds into a
  rank-1 PSUM update bo2_f * esum_q applied by a tiny matmul.
- Output is produced in [F, NQ] (transposed) layout; the host transposes back.
- Host passes x^T in fp8 per core, rotated so the core's own query columns are
  always columns 0:2048 -> identical SPMD graph on all cores.
- Scores are computed transposed (S^T = K @ Q^T, layout [j, q]); exp runs
  1024-wide over score PAIRS (two PSUM banks per ACTIVATE) to amortize the
  ~150-cycle ACT init.
- Softmax denominators: E pair-tiles are accumulated elementwise on Vector
  (5/8) and GpSimd (3/8); at q-block end they are reduced over partitions by
  four of=512 ones-matmuls into a [1,512] PSUM row, reciprocal'd on DVE, and
  broadcast back to [128,512] with a tiny fp32 matmul; final scaling is a DVE
  tensor_tensor multiply straight out of PSUM.
"""

import math
import sys

import numpy as np

for _p in ("/opt/trn_rl_repo", "/opt/pypackages"):
    if _p not in sys.path:
        sys.path.append(_p)

import ml_dtypes

N = 16384
F = 512
MD = 128
P = 128
NCORES = 8
NQ = N // NCORES      # 2048 query rows per core
QB = 512              # q-block (one PSUM bank of fp32)
NQB = NQ // QB        # 4
JT = 128              # j (key) tile
NJT = N // JT         # 128
FK = F // P           # 4 contraction tiles over features
CH = 512              # xt streaming chunk (j columns)
NCH = N // CH         # 32
GK = 16               # j-tiles per SBUF super-group
NG = NJT // GK        # 8
WS = 16.0             # host-side fp8 weight pre-scale (Wq/Wk)
WS2 = 32.0            # host-side fp8 pre-scale for W2 = Wv@Wo
SCALE = 1.0 / math.sqrt(MD) / (WS * WS)

_BF16 = ml_dtypes.bfloat16
_FP8 = ml_dtypes.float8_e4m3fn


def _build():
    import concourse.bass as bass  # noqa: F401
    import concourse.tile as tile
    from concourse import bacc, mybir

    f32 = mybir.dt.float32
    bf16 = mybir.dt.bfloat16
    fp8 = mybir.dt.float8e4
    DR = mybir.MatmulPerfMode.DoubleRow
    AF = mybir.ActivationFunctionType
    ALU = mybir.AluOpType

    nc = bacc.Bacc("TRN2", target_bir_lowering=False, debug=False,
                   num_devices=NCORES)

    xt = nc.declare_dram_parameter("xt", [F, N], fp8, isOutput=False)
    wq = nc.declare_dram_parameter("wq", [F, MD], fp8, isOutput=False)
    wk = nc.declare_dram_parameter("wk", [F, MD], fp8, isOutput=False)
    w2 = nc.declare_dram_parameter("w2", [F, F], fp8, isOutput=False)
    bq = nc.declare_dram_parameter("bq", [MD, 1], f32, isOutput=False)
    bo2 = nc.declare_dram_parameter("bo2", [1, F], f32, isOutput=False)
    out = nc.declare_dram_parameter("out", [F, NQ], f32, isOutput=True)

    with tile.TileContext(nc) as tc:
        with (
            tc.tile_pool(name="persist", bufs=1) as pp,
            tc.tile_pool(name="stream", bufs=4) as sp,
            tc.tile_pool(name="work", bufs=3) as wkp,
            tc.tile_pool(name="pssc", bufs=2, space="PSUM") as ps_sc,
            tc.tile_pool(name="pso", bufs=4, space="PSUM") as ps_o,
        ):
            # ---- persistent constants (vector/scalar DMA queues so the
            # gpsimd xt stream is not serialized behind them) --------------
            wq_a = pp.tile([P, FK, MD], fp8, tag="wqa")
            wk_a = pp.tile([P, FK, MD], fp8, tag="wka")
            w2_a = pp.tile([P, FK, F], fp8, tag="w2a")
            for k in range(FK):
                nc.sync.dma_start(out=wk_a[:, k, :], in_=wk[k * P:(k + 1) * P, :])
                nc.sync.dma_start(out=w2_a[:, k, :], in_=w2[k * P:(k + 1) * P, :])
            for k in range(FK):
                nc.scalar.dma_start(out=wq_a[:, k, :], in_=wq[k * P:(k + 1) * P, :])
            bq_t = pp.tile([MD, 1], f32, tag="bq")
            nc.scalar.dma_start(out=bq_t[:], in_=bq[:])
            # bo2 as bf16 stationary rows [1, FK, 128] for the rank-1 update
            bo2_f = pp.tile([1, F], f32, tag="bo2f")
            nc.scalar.dma_start(out=bo2_f[:], in_=bo2[:])
            bo2_b = pp.tile([1, FK, P], bf16, tag="bo2b")
            nc.vector.tensor_copy(bo2_b[:].rearrange("o k p -> o (k p)"),
                                  bo2_f[:])
            ones_f = pp.tile([P, 1], bf16, tag="ones")
            nc.vector.memset(ones_f[:], 1.0)
            ones1_f32 = pp.tile([1, P], f32, tag="ones1")
            nc.vector.memset(ones1_f32[:], 1.0)
            id2 = pp.tile([P, 2, P], fp8, tag="id2")
            from concourse.masks import make_identity
            make_identity(nc, id2[:, 0, :])
            make_identity(nc, id2[:, 1, :])

            # ---- persistent activations -----------------------------------
            ktg = [pp.tile([P, GK * JT], bf16, tag=f"ktg{g}", name=f"ktg{g}")
                   for g in range(NG)]
            vg = [pp.tile([P, GK * F], fp8, tag=f"vg{g}", name=f"vg{g}")
                  for g in range(NG)]
            qt = pp.tile([P, NQ], bf16, tag="qt")

            # ---- PE warmup during the initial DMA wait (HAM un-throttle) --
            warm_ps = ps_sc.tile([P, 2, P], f32, tag="sc", name="warm_ps")
            for wi in range(20):
                nc.tensor.matmul(warm_ps[:, 0, :], id2[:, 0, :], id2[:, 0, :],
                                 start=(wi == 0), stop=(wi == 19))
            warm_s = pp.tile([P, P], bf16, tag="warms")
            nc.scalar.copy(warm_s[:], warm_ps[:, 0, :])

            # ---- prologue: project Q^T, K^T, V' (fp8 DoubleRow) -----------
            for ch in range(NCH):
                xtc = sp.tile([P, FK, CH], fp8, tag="xtc")
                xt4 = xt.rearrange("(k p) n -> p k n", p=P)
                dma_eng = nc.gpsimd if ch % 2 == 0 else nc.sync
                if ch < 2:
                    for k in range(FK):
                        dma_eng.dma_start(
                            out=xtc[:, k, :],
                            in_=xt[k * P:(k + 1) * P, ch * CH:(ch + 1) * CH])
                else:
                    dma_eng.dma_start(
                        out=xtc[:], in_=xt4[:, :, ch * CH:(ch + 1) * CH])
                g, off = ch // 4, (ch % 4) * CH
                pk = ps_sc.tile([P, 2, CH], f32, tag="sc", name="pk")
                for h in range(2):
                    nc.tensor.matmul(pk[:, 0, :], wk_a[:, 2 * h:2 * h + 2, :],
                                     xtc[:, 2 * h:2 * h + 2, :],
                                     start=(h == 0), stop=(h == 1), perf_mode=DR)
                nc.scalar.copy(ktg[g][:, off:off + CH], pk[:, 0, :])
                if ch < NQ // CH:
                    pq = ps_sc.tile([P, 2, CH], f32, tag="sc", name="pq")
                    for h in range(2):
                        nc.tensor.matmul(pq[:, 0, :], wq_a[:, 2 * h:2 * h + 2, :],
                                         xtc[:, 2 * h:2 * h + 2, :],
                                         start=(h == 0), stop=(h == 1),
                                         perf_mode=DR)
                    nc.scalar.activation(qt[:, ch * CH:(ch + 1) * CH],
                                         pq[:, 0, :],
                                         AF.Identity, bias=bq_t[:], scale=1.0)
                for js in range(CH // JT):
                    jt_g = ch * (CH // JT) + js
                    voff = (jt_g % GK) * F
                    pv = ps_o.tile([P, F], f32, tag="oacc", name="pv")
                    for h in range(2):
                        nc.tensor.matmul(
                            pv[:], xtc[:, 2 * h:2 * h + 2, js * JT:(js + 1) * JT],
                            w2_a[:, 2 * h:2 * h + 2, :],
                            start=(h == 0), stop=(h == 1), perf_mode=DR)
                    if jt_g % 2 == 0:
                        nc.vector.tensor_copy(vg[jt_g // GK][:, voff:voff + F],
                                              pv[:])
                    else:
                        nc.scalar.copy(vg[jt_g // GK][:, voff:voff + F], pv[:])

            # ---- attention: flat pipeline over all (q-block, key-pair) ----
            # E-sum pairs split 5:3 between DVE and GpSimd. Score matmuls
            # write [P,2,QB] pair tiles; one 1024-wide exp per pair. The last
            # pair's e-sum add goes to DVE so the epilogue chain (es matmuls
            # -> recip -> broadcast -> rank-1 bias -> scaled writeback) sits
            # right behind it in the DVE FIFO.
            NP2 = NJT // 2
            SUMS_PAT = {0: "G", 1: "D", 2: "G", 3: "D",
                        4: "D", 5: "G", 6: "D", 7: "D"}

            def scores_pair(gp):
                qbb, p_i = gp // NP2, gp % NP2
                jt0 = 2 * p_i
                g, r0 = jt0 // GK, jt0 % GK
                psc = ps_sc.tile([P, 2, QB], f32, tag="sc", name="psc")
                for h in range(2):
                    nc.tensor.matmul(psc[:, h, :],
                                     ktg[g][:, (r0 + h) * JT:(r0 + h + 1) * JT],
                                     qt[:, qbb * QB:(qbb + 1) * QB],
                                     start=True, stop=True)
                return psc

            pending = {j: scores_pair(j) for j in range(2)}
            state = {}

            def epilogue(st):
                # Inline at the last pair of a q-block: drains po banks fast
                # so the next q-block's O-acc can claim them.
                po = st["po"]
                acc_d, acc_g = st["acc_d"], st["acc_g"]
                es_ps = st["es_ps"]
                srcs = [acc_d[:, 0, :], acc_d[:, 1, :],
                        acc_g[:, 0, :], acc_g[:, 1, :]]
                for si, s in enumerate(srcs):
                    nc.tensor.matmul(es_ps[:1, 0, :], ones_f[:], s,
                                     start=(si == 0), stop=(si == 3))
                recip_sb = wkp.tile([1, QB], f32, tag="recip", bufs=2,
                                    name="recip_sb")
                nc.vector.reciprocal(recip_sb[:], es_ps[:1, 0, :])
                es_sb = wkp.tile([1, QB], bf16, tag="essb", bufs=2,
                                 name="es_sb")
                nc.vector.tensor_copy(es_sb[:], es_ps[:1, 0, :])
                # broadcast 1/es across 128 partitions (fp32 matmul, of=512)
                nc.tensor.matmul(es_ps[:, 1, :], ones1_f32[:], recip_sb[:],
                                 start=True, stop=True)
                recip_bc = wkp.tile([P, QB], f32, tag="recipbc", bufs=2,
                                    name="recip_bc")
                nc.scalar.copy(recip_bc[:], es_ps[:, 1, :])
                for ft in range(FK):
                    # po[ft] += bo2_f * es_q  (rank-1; closes the accum group)
                    nc.tensor.matmul(po[ft][:], bo2_b[:, ft, :], es_sb[:],
                                     start=False, stop=True)
                for ft in range(FK):
                    out_t = wkp.tile([P, QB], f32, tag="outt", bufs=4,
                                     name="out_t")
                    nc.vector.tensor_tensor(out_t[:], po[ft][:],
                                            recip_bc[:], ALU.mult)
                    row0 = ft * P
                    col0 = st["qb"] * QB
                    nc.sync.dma_start(out=out[row0:row0 + P, col0:col0 + QB],
                                      in_=out_t[:])

            for gp_i in range(NQB * NP2):
                qb, p_i = gp_i // NP2, gp_i % NP2
                if p_i == 0:
                    state = {
                        "qb": qb,
                        "po": [ps_o.tile([P, QB], f32, tag="oacc", name="oacc")
                               for _ in range(FK)],
                        "acc_d": wkp.tile([P, 2, QB], bf16, tag="accd", bufs=2,
                                          name="acc_d"),
                        "acc_g": wkp.tile([P, 2, QB], bf16, tag="accg", bufs=2,
                                          name="acc_g"),
                        "seen": {"d": False, "g": False},
                    }
                jt0 = 2 * p_i
                g, r0 = jt0 // GK, jt0 % GK
                psc = pending.pop(gp_i)
                etp = wkp.tile([P, 2, QB], fp8, tag="et", bufs=6)
                nc.scalar.activation(etp[:], psc[:], AF.Exp, scale=SCALE)
                last = p_i == NP2 - 1
                if last:
                    # allocate the epilogue's psum tile BEFORE seeding the
                    # next pair so it lands on the slot whose previous user
                    # (this pair's psc) is already being drained by exp.
                    state["es_ps"] = ps_sc.tile([P, 2, QB], f32, tag="sc",
                                                name="es_ps")
                nxt = gp_i + 2
                if nxt < NQB * NP2:
                    pending[nxt] = scores_pair(nxt)
                kind = SUMS_PAT[p_i % 8]
                eng, acc, key = ((nc.vector, state["acc_d"], "d")
                                 if kind == "D"
                                 else (nc.gpsimd, state["acc_g"], "g"))
                if not state["seen"][key]:
                    eng.tensor_copy(acc[:], etp[:])
                    state["seen"][key] = True
                else:
                    eng.tensor_tensor(acc[:], acc[:], etp[:], ALU.add)
                vg4 = vg[g].rearrange("p (t h f) -> p t h f", h=2, f=F)
                for ft in range(FK):
                    nc.tensor.matmul(
                        state["po"][ft][:],
                        vg4[:, r0 // 2, :, ft * P:(ft + 1) * P],
                        etp[:], start=(p_i == 0), stop=False,
                        perf_mode=DR)
                if last:
                    epilogue(state)

    nc.compile()
    return nc


_CACHED = {}


def _get_nc():
    if "nc" not in _CACHED:
        _CACHED["nc"] = _build()
    return _CACHED["nc"]


def _make_in_maps(x, Wq, bq, Wk, bk, Wv, bv, Wo, bo):
    x = np.asarray(x, dtype=np.float32)
    xt_full = np.ascontiguousarray(x.T)                     # [F, N] f32
    wq_8 = (WS * np.asarray(Wq, np.float32)).astype(_FP8)
    wk_8 = (WS * np.asarray(Wk, np.float32)).astype(_FP8)
    w2_f = np.asarray(Wv, np.float64) @ np.asarray(Wo, np.float64)
    w2_8 = (WS2 * w2_f).astype(np.float32).astype(_FP8)
    bq_h = (WS * np.asarray(bq, np.float32)).reshape(MD, 1).astype(np.float32)
    bo_p = (np.asarray(bv, np.float64) @ np.asarray(Wo, np.float64)
            + np.asarray(bo, np.float64)).astype(np.float32).reshape(1, F)
    # the kernel's PSUM accumulates (WS2 * V') E; bo2 rides inside the same
    # pre-scaled accumulator via the rank-1 matmul whose es_sb operand is the
    # raw (unscaled) esum, so pre-multiply bo2 by WS2 to match, then the final
    # multiply by recip = 1/esum and the host divide by WS2 undo everything.
    bo2_h = (WS2 * bo_p).astype(np.float32)

    in_maps = []
    for c in range(NCORES):
        s = c * NQ
        xt_rot = np.concatenate([xt_full[:, s:], xt_full[:, :s]], axis=1)
        in_maps.append({
            "xt": np.ascontiguousarray(xt_rot).astype(_FP8),
            "wq": wq_8, "wk": wk_8, "w2": w2_8,
            "bq": bq_h, "bo2": bo2_h,
        })
    return in_maps


def _gather(res):
    outs = []
    for c in range(NCORES):
        o = res.results[c]["out"]           # [F, NQ] f32, scaled by WS2
        outs.append(np.ascontiguousarray(o.T) / WS2)
    return np.concatenate(outs, axis=0)


def kernel(x, Wq, bq, Wk, bk, Wv, bv, Wo, bo):
    from concourse.bass_utils import run_bass_kernel_spmd

    in_maps = _make_in_maps(x, Wq, bq, Wk, bk, Wv, bv, Wo, bo)
    nc = _get_nc()
    res = run_bass_kernel_spmd(nc, in_maps, core_ids=list(range(NCORES)))
    return _gather(res)


def run_traced(x, Wq, bq, Wk, bk, Wv, bv, Wo, bo):
    """Like kernel() but with NTFF tracing; returns (output, exec_time_ns)."""
    from concourse.bass_utils import run_bass_kernel_spmd

    try:
        import ntff_shim
        ntff_shim.install()
    except ImportError:
        pass
    in_maps = _make_in_maps(x, Wq, bq, Wk, bk, Wv, bv, Wo, bo)
    nc = _get_nc()
    res = run_bass_kernel_spmd(nc, in_maps, core_ids=list(range(NCORES)),
                               trace=True)
    return _gather(res), res.exec_time_ns
